# revision 10
# baseline (speedup 1.0000x reference)
import sys

for p in ("/opt/trn_rl_repo", "/opt/trn_rl_repo/concourse"):
    if p not in sys.path:
        sys.path.insert(0, p)

import numpy as np

import concourse.bacc as bacc
import concourse.bass as bass
import concourse.mybir as mybir
import concourse.tile as tile
from concourse.bass_utils import run_bass_kernel_spmd

LOG2PI = float(np.log(2.0 * np.pi))

N, T, D = 16, 2048, 2
NCORES = 8
SEQ_PER_CORE = N // NCORES  # 2
P = 128                     # strip height / partitions
NSTRIP = T // P             # 16
CHUNK = 512                 # psum bank width (f32)
MASKNEG = -1.0e30

_cached = {}


def _build_nc():
    nc = bacc.Bacc(None, target_bir_lowering=False)
    f32 = mybir.dt.float32

    LR_d = nc.dram_tensor("LR", [SEQ_PER_CORE, 4, 2 * T], f32, kind="ExternalInput")
    M_d = nc.dram_tensor("MASKADD", [P, P], f32, kind="ExternalInput")
    O_d = nc.dram_tensor("out", [SEQ_PER_CORE, T], f32, kind="ExternalOutput")

    with tile.TileContext(nc) as tc:
        with (
            tc.tile_pool(name="const", bufs=1) as cpool,
            tc.tile_pool(name="io", bufs=2) as iopool,
            tc.tile_pool(name="work", bufs=4) as wpool,
            tc.tile_pool(name="stat", bufs=4) as spool,
            tc.tile_pool(name="psum", bufs=4, space=bass.MemorySpace.PSUM) as ppool,
        ):
            maskadd = cpool.tile([P, P], f32)
            nc.sync.dma_start(maskadd[:], M_d[:])
            junk = cpool.tile([P, 1], f32)

            for s in range(SEQ_PER_CORE):
                LRt = iopool.tile([4, 2 * T], f32, tag="LR")
                nc.sync.dma_start(LRt[:], LR_d[s])

                for k in range(NSTRIP):
                    i0 = k * P
                    # full causal chunks [0, i0), then the diagonal P-wide block
                    chunks = [(j0, min(CHUNK, i0 - j0)) for j0 in range(0, i0, CHUNK)]
                    nch = len(chunks) + 1
                    partials = spool.tile([P, 8], f32, tag="partials")
                    lhsT = LRt[:, i0:i0 + P]

                    for c, (j0, w) in enumerate(chunks):
                        ps = ppool.tile([P, CHUNK], f32, tag="ps")
                        e = wpool.tile([P, CHUNK], f32, tag="e")
                        nc.tensor.matmul(ps[:, :w], lhsT, LRt[:, T + j0:T + j0 + w])
                        nc.scalar.activation(
                            e[:, :w], ps[:, :w],
                            mybir.ActivationFunctionType.Exp,
                            accum_out=partials[:, c:c + 1],
                        )

                    # diagonal block with strict lower-triangular additive mask
                    psd = ppool.tile([P, CHUNK], f32, tag="ps")
                    argd = wpool.tile([P, P], f32, tag="argd")
                    ed = wpool.tile([P, P], f32, tag="ed")
                    nc.tensor.matmul(psd[:, :P], lhsT, LRt[:, T + i0:T + i0 + P])
                    nc.vector.tensor_copy(argd[:], psd[:, :P])
                    nc.vector.tensor_add(argd[:], argd[:], maskadd[:])
                    nc.scalar.activation(
                        ed[:], argd[:],
                        mybir.ActivationFunctionType.Exp,
                        accum_out=partials[:, nch - 1:nch],
                    )

                    acc = spool.tile([P, 1], f32, tag="acc")
                    lnA = spool.tile([P, 1], f32, tag="lnA")
                    nc.vector.tensor_reduce(
                        acc[:], partials[:, :nch],
                        mybir.AxisListType.X, mybir.AluOpType.add,
                    )
                    nc.scalar.activation(
                        lnA[:], acc[:], mybir.ActivationFunctionType.Ln,
                    )
                    nc.sync.dma_start(O_d[s, i0:i0 + P], lnA[:, 0])
    nc.compile()
    return nc


def kernel(event_times, spatial_locations, input_mask, mu0, logstd0,
           coeff_decay, spatial_logstd):
    t = np.asarray(event_times, np.float64)            # (N, T)
    x = np.asarray(spatial_locations, np.float32)      # (N, T, D)
    m = np.asarray(input_mask, np.float32)             # (N, T)
    mu0 = float(np.asarray(mu0)); ls0 = float(np.asarray(logstd0))
    cd = float(np.asarray(coeff_decay)); sls = float(np.asarray(spatial_logstd))

    sp = float(np.log1p(np.exp(cd)))                   # softplus
    c2 = float(np.exp(-2.0 * sls))
    dconst = D * (2.0 * sls + LOG2PI)

    sq = np.sum(x.astype(np.float64) ** 2, axis=-1)    # (N, T)
    a = t / sp                                         # (N, T)
    u = (-0.5 * c2 * sq - a - 0.5 * dconst).astype(np.float32)
    v = (-0.5 * c2 * sq + a).astype(np.float32)

    ones = np.ones((N, T), np.float32)
    Lrows = np.stack([x[:, :, 0], x[:, :, 1], ones, u], axis=1)            # (N,4,T)
    Rrows = np.stack([c2 * x[:, :, 0], c2 * x[:, :, 1], v, ones], axis=1)  # (N,4,T)
    LR = np.concatenate([Lrows, Rrows], axis=2)                            # (N,4,2T)

    ii = np.arange(P)
    maskadd = np.where(ii[:, None] > ii[None, :], 0.0, MASKNEG).astype(np.float32)

    if "nc" not in _cached:
        _cached["nc"] = _build_nc()
    nc = _cached["nc"]

    in_maps = []
    for c in range(NCORES):
        s0, s1 = c * SEQ_PER_CORE, (c + 1) * SEQ_PER_CORE
        in_maps.append({
            "LR": np.ascontiguousarray(LR[s0:s1]),
            "MASKADD": maskadd,
        })
    res = run_bass_kernel_spmd(nc, in_maps, core_ids=list(range(NCORES)))
    lnA = np.concatenate([r["out"] for r in res.results], axis=0)  # (N, T)

    # denominator: B[i] = logsumexp_{j<i}(a_j) - a_i, exclusive cumulative lse
    cum = np.logaddexp.accumulate(a, axis=1)           # (N, T) f64
    B = np.empty_like(a)
    B[:, 1:] = cum[:, :-1] - a[:, 1:]
    B[:, 0] = 0.0

    loglik = (lnA.astype(np.float64) - B) * m

    tmp0 = (x[:, 0].astype(np.float64) - mu0) * np.exp(-ls0)
    loglik0 = np.sum(-0.5 * (tmp0 * tmp0 + 2.0 * ls0 + LOG2PI), axis=-1)  # (N,)

    out = np.concatenate([loglik0[:, None], loglik[:, 1:]], axis=1)
    return out.astype(np.float32)


# revision 12
# speedup vs baseline: 2.0704x; 2.0704x over previous
import sys

for p in ("/opt/trn_rl_repo", "/opt/trn_rl_repo/concourse"):
    if p not in sys.path:
        sys.path.insert(0, p)

import numpy as np

import concourse.bacc as bacc
import concourse.bass as bass
import concourse.mybir as mybir
import concourse.tile as tile
from concourse.bass_utils import run_bass_kernel_spmd

LOG2PI = float(np.log(2.0 * np.pi))

N, T, D = 16, 2048, 2
NCORES = 8
SEQ_PER_CORE = N // NCORES  # 2
P = 128                     # strip height / partitions
NSTRIP = T // P             # 16
CHUNK = 512                 # psum bank width (f32)
MASKNEG = -1.0e30

_cached = {}


def _build_nc():
    nc = bacc.Bacc(None, target_bir_lowering=False)
    f32 = mybir.dt.float32

    LR_d = nc.dram_tensor("LR", [SEQ_PER_CORE, 4, 2 * T], f32, kind="ExternalInput")
    M_d = nc.dram_tensor("MASKADD", [P, P], f32, kind="ExternalInput")
    O_d = nc.dram_tensor("out", [SEQ_PER_CORE, T], f32, kind="ExternalOutput")

    with tile.TileContext(nc) as tc:
        with (
            tc.tile_pool(name="const", bufs=1) as cpool,
            tc.tile_pool(name="io", bufs=2) as iopool,
            tc.tile_pool(name="work", bufs=4) as wpool,
            tc.tile_pool(name="stat", bufs=4) as spool,
            tc.tile_pool(name="psum", bufs=4, space=bass.MemorySpace.PSUM) as ppool,
        ):
            maskadd = cpool.tile([P, P], f32)
            nc.sync.dma_start(maskadd[:], M_d[:])
            junk = cpool.tile([P, 1], f32)

            for s in range(SEQ_PER_CORE):
                LRt = iopool.tile([4, 2 * T], f32, tag="LR")
                nc.sync.dma_start(LRt[:], LR_d[s])

                for k in range(NSTRIP):
                    i0 = k * P
                    # full causal chunks [0, i0), then the diagonal P-wide block
                    chunks = [(j0, min(CHUNK, i0 - j0)) for j0 in range(0, i0, CHUNK)]
                    nch = len(chunks) + 1
                    partials = spool.tile([P, 8], f32, tag="partials")
                    lhsT = LRt[:, i0:i0 + P]

                    for c, (j0, w) in enumerate(chunks):
                        ps = ppool.tile([P, CHUNK], f32, tag="ps")
                        e = wpool.tile([P, CHUNK], f32, tag="e")
                        nc.tensor.matmul(ps[:, :w], lhsT, LRt[:, T + j0:T + j0 + w])
                        nc.scalar.activation(
                            e[:, :w], ps[:, :w],
                            mybir.ActivationFunctionType.Exp,
                            accum_out=partials[:, c:c + 1],
                        )

                    # diagonal block with strict lower-triangular additive mask
                    psd = ppool.tile([P, CHUNK], f32, tag="ps")
                    argd = wpool.tile([P, P], f32, tag="argd")
                    ed = wpool.tile([P, P], f32, tag="ed")
                    nc.tensor.matmul(psd[:, :P], lhsT, LRt[:, T + i0:T + i0 + P])
                    nc.vector.tensor_copy(argd[:], psd[:, :P])
                    nc.vector.tensor_add(argd[:], argd[:], maskadd[:])
                    nc.scalar.activation(
                        ed[:], argd[:],
                        mybir.ActivationFunctionType.Exp,
                        accum_out=partials[:, nch - 1:nch],
                    )

                    acc = spool.tile([P, 1], f32, tag="acc")
                    lnA = spool.tile([P, 1], f32, tag="lnA")
                    nc.vector.tensor_reduce(
                        acc[:], partials[:, :nch],
                        mybir.AxisListType.X, mybir.AluOpType.add,
                    )
                    nc.scalar.activation(
                        lnA[:], acc[:], mybir.ActivationFunctionType.Ln,
                    )
                    nc.sync.dma_start(O_d[s, i0:i0 + P], lnA[:, 0])
    nc.compile()
    return nc


def _get_runner():
    """Build the Bass program and a cached jitted shard_map executor once.

    Mirrors bass2jax.run_bass_via_pjrt, but keeps the jitted callable
    across kernel() invocations to avoid per-call retracing.
    """
    if "runner" in _cached:
        return _cached["runner"]

    import jax
    from jax.sharding import Mesh, PartitionSpec
    from jax.experimental.shard_map import shard_map
    import concourse.bass2jax as b2j
    import concourse.mybir as mb

    nc = _build_nc()
    b2j.install_neuronx_cc_hook()

    partition_name = nc.partition_id_tensor.name if nc.partition_id_tensor else None
    in_names, out_names, out_avals = [], [], []
    for alloc in nc.m.functions[0].allocations:
        if not isinstance(alloc, mb.MemoryLocationSet):
            continue
        name = alloc.memorylocations[0].name
        if alloc.kind == "ExternalInput":
            if name != partition_name:
                in_names.append(name)
        elif alloc.kind == "ExternalOutput":
            shape = tuple(alloc.tensor_shape)
            dtype = mb.dt.np(alloc.dtype)
            out_names.append(name)
            out_avals.append(jax.core.ShapedArray(shape, dtype))
    n_params = len(in_names)
    n_outs = len(out_avals)
    all_in_names = in_names + out_names
    if partition_name is not None:
        all_in_names = all_in_names + [partition_name]
    donate = tuple(range(n_params, n_params + n_outs))

    def _body(*args):
        operands = list(args)
        if partition_name is not None:
            operands.append(b2j.partition_id_tensor())
        outs = b2j._bass_exec_p.bind(
            *operands,
            out_avals=tuple(out_avals),
            in_names=tuple(all_in_names),
            out_names=tuple(out_names),
            lowering_input_output_aliases=(),
            sim_require_finite=True,
            sim_require_nnan=True,
            nc=nc,
        )
        return tuple(outs)

    devices = jax.devices()[:NCORES]
    mesh = Mesh(np.asarray(devices), ("core",))
    in_specs = (PartitionSpec("core"),) * (n_params + n_outs)
    out_specs = (PartitionSpec("core"),) * n_outs
    sharded = jax.jit(
        shard_map(_body, mesh=mesh, in_specs=in_specs, out_specs=out_specs,
                  check_rep=False),
        donate_argnums=donate, keep_unused=True,
    )
    _cached["runner"] = (sharded, in_names, out_names, out_avals)
    return _cached["runner"]


def _run_device(LR, maskadd):
    sharded, in_names, out_names, out_avals = _get_runner()
    per_name = {
        "LR": LR.reshape(N, 4, 2 * T).astype(np.float32, copy=False),
        "MASKADD": np.broadcast_to(maskadd, (NCORES, P, P)).reshape(NCORES * P, P),
    }
    concat_in = [np.ascontiguousarray(per_name[nm]) for nm in in_names]
    concat_zeros = [
        np.zeros((NCORES * a.shape[0], *a.shape[1:]), a.dtype) for a in out_avals
    ]
    out_arrs = sharded(*concat_in, *concat_zeros)
    i = out_names.index("out")
    return np.asarray(out_arrs[i]).reshape(N, T)


def kernel(event_times, spatial_locations, input_mask, mu0, logstd0,
           coeff_decay, spatial_logstd):
    t = np.asarray(event_times, np.float64)            # (N, T)
    x = np.asarray(spatial_locations, np.float32)      # (N, T, D)
    m = np.asarray(input_mask, np.float32)             # (N, T)
    mu0 = float(np.asarray(mu0)); ls0 = float(np.asarray(logstd0))
    cd = float(np.asarray(coeff_decay)); sls = float(np.asarray(spatial_logstd))

    sp = float(np.log1p(np.exp(cd)))                   # softplus
    c2 = float(np.exp(-2.0 * sls))
    dconst = D * (2.0 * sls + LOG2PI)

    sq = np.sum(x.astype(np.float64) ** 2, axis=-1)    # (N, T)
    a = t / sp                                         # (N, T)
    u = (-0.5 * c2 * sq - a - 0.5 * dconst).astype(np.float32)
    v = (-0.5 * c2 * sq + a).astype(np.float32)

    ones = np.ones((N, T), np.float32)
    Lrows = np.stack([x[:, :, 0], x[:, :, 1], ones, u], axis=1)            # (N,4,T)
    Rrows = np.stack([c2 * x[:, :, 0], c2 * x[:, :, 1], v, ones], axis=1)  # (N,4,T)
    LR = np.concatenate([Lrows, Rrows], axis=2)                            # (N,4,2T)

    ii = np.arange(P)
    maskadd = np.where(ii[:, None] > ii[None, :], 0.0, MASKNEG).astype(np.float32)

    lnA = _run_device(LR, maskadd)  # (N, T)

    # denominator: B[i] = logsumexp_{j<i}(a_j) - a_i, exclusive cumulative lse
    cum = np.logaddexp.accumulate(a, axis=1)           # (N, T) f64
    B = np.empty_like(a)
    B[:, 1:] = cum[:, :-1] - a[:, 1:]
    B[:, 0] = 0.0

    loglik = (lnA.astype(np.float64) - B) * m

    tmp0 = (x[:, 0].astype(np.float64) - mu0) * np.exp(-ls0)
    loglik0 = np.sum(-0.5 * (tmp0 * tmp0 + 2.0 * ls0 + LOG2PI), axis=-1)  # (N,)

    out = np.concatenate([loglik0[:, None], loglik[:, 1:]], axis=1)
    return out.astype(np.float32)
